# revision 27
# baseline (speedup 1.0000x reference)
"""GNN message-passing kernel for 8 Trainium2 NeuronCores.

Strategy (node-sharded, zero collectives), v5 (bf16, host-built
selection matrices):
  - Pad nodes to 50176 = 8 cores x 49 tiles x 128 slots. A host-side
    "snake deal" assigns nodes to tiles balancing per-tile edge counts;
    within each tile, nodes are packed into H fixed-size windows (~13
    nodes each) such that every window's edge count is <= 128 in all 4
    streams (mi/mo x lo/hi half). The node's partition within the tile
    is its window offset + position (the MLP is pointwise, so any node
    permutation is legal; it is undone on the host at the end).
  - Edges are duplicated per direction: mi keyed by dst (gather x[src]),
    mo keyed by src (gather x[dst]); per (tile, half) they are grouped
    by the key node's window into H blocks of <= 128 edges (one padded
    128-slot block per window).
  - On-core: rolling dma_gather chunks bring bf16 x rows into SBUF.
    Aggregation per block is a single PE matmul psum[:, w0:w1] +=
    Y^T @ S where S [128 slots, ~13 window nodes] is a host-built bf16
    selection matrix carrying the edge weights - no on-chip one-hot
    construction at all (v2-v3 burned 300-400ns of DVE per block there).
    Window columns are disjoint per stream, so lo-half blocks write
    with start=True and hi-half blocks accumulate.
  - All metadata (gather indices, xT) is bulk-loaded up front; S is
    streamed in 8-tile double-buffered groups; output is buffered in
    SBUF (bf16) and stored in 4-tile groups.
  - The 4-layer MLP runs feature-major in bf16, batched over 4 tiles
    per matmul group (one psum bank wide).
"""

import os
import sys

sys.path.insert(0, "/opt/trn_rl_repo")

import numpy as np
import ml_dtypes

from concourse import bass, bacc, mybir, tile
from concourse import bass_utils

N = 50000
E = 800000
D = 128
N_CORES = 8
T_CORE = 49                      # tiles per core
T_TOT = N_CORES * T_CORE         # 392 tiles
NPAD = T_TOT * 128               # 50176
HALF = NPAD // 2                 # 25088 (int16 index limit is 32767)
CHUNK = 8                        # gather blocks per dma_gather call
LA = 8                           # gather lookahead in tiles
GM = 4                           # tiles per batched-MLP group (psum bank cap)
SG = 4                           # tiles per rolling S-load group

bf16_np = ml_dtypes.bfloat16

f32 = mybir.dt.float32
bf16 = mybir.dt.bfloat16
i16 = mybir.dt.int16

LAST_RESULTS = None              # BassKernelResults of the last run


def _register_ntff_hook():
    """Make trace=True work under axon by registering the NTFF profile
    hook that the agent image's antenv package lacks."""
    import types, ctypes, contextlib

    if "antenv.axon_hooks" in sys.modules:
        return
    so_path = "/opt/axon/libaxon_pjrt.so"
    if not os.path.exists(so_path):
        return
    try:
        lib = ctypes.CDLL(so_path)
        if not hasattr(lib, "axon_start_nrt_profile"):
            return
        lib.axon_start_nrt_profile.argtypes = [
            ctypes.POINTER(ctypes.c_int64), ctypes.c_size_t]
        lib.axon_start_nrt_profile.restype = ctypes.c_int64
        lib.axon_stop_nrt_profile.argtypes = [ctypes.c_char_p]
        lib.axon_stop_nrt_profile.restype = ctypes.c_int64

        @contextlib.contextmanager
        def _hook(output_dir, device_ids):
            import jax
            jax.devices()
            if device_ids:
                ids = (ctypes.c_int64 * len(device_ids))(*device_ids)
                rc = lib.axon_start_nrt_profile(ids, len(device_ids))
            else:
                rc = lib.axon_start_nrt_profile(None, 0)
            if rc != 0:
                raise RuntimeError(f"axon_start_nrt_profile rc={rc}")
            try:
                yield
            finally:
                n = lib.axon_stop_nrt_profile(str(output_dir).encode())
                print(f"profile: {n} file(s) -> {output_dir}", file=sys.stderr)

        mod = types.ModuleType("antenv.axon_hooks")
        mod.get_axon_ntff_profile_hook = lambda: _hook
        sys.modules["antenv.axon_hooks"] = mod
    except OSError:
        pass


def _win_bounds(H):
    b = [round(128 * i / H) for i in range(H + 1)]
    return b, [b[i + 1] - b[i] for i in range(H)]


def _snake_slots(C):
    """Assign each padded node to a global tile, balancing total edge
    counts across tiles (snake deal on the 4-cell sum)."""
    tot = C.sum(1)
    rank = np.argsort(-tot, kind="stable")
    seq = np.arange(NPAD)
    rounds = seq // T_TOT
    k = seq % T_TOT
    tile_seq = np.where(rounds % 2 == 0, k, T_TOT - 1 - k).astype(np.int32)
    gtile = np.empty(NPAD, np.int32)
    gtile[rank] = tile_seq
    return gtile


def _pack_windows(C, gtile, H):
    """Within each tile, pack its 128 nodes into H fixed-size windows
    so that every window's edge count is <= 128 in each of the 4
    streams (greedy with randomized restarts). Returns (win, posn) per
    node, or (None, None) if packing fails for any tile."""
    bounds, sizes = _win_bounds(H)
    sizes = np.array(sizes)
    win = np.empty(NPAD, np.int32)
    posn = np.empty(NPAD, np.int32)
    rng = np.random.default_rng(7)

    def pack_tile(V, order):
        loads = np.zeros((H, 4), np.int64)
        cnt = np.zeros(H, np.int64)
        wi = np.empty(len(V), np.int32)
        pi = np.empty(len(V), np.int32)
        for i in order:
            v = V[i]
            sc = (loads + v).max(1) + ((loads + v) > 128).any(1) * 100000
            sc = np.where(cnt >= sizes, 10 ** 9, sc)
            b = int(np.argmin(sc))
            wi[i] = b
            pi[i] = cnt[b]
            loads[b] += v
            cnt[b] += 1
        return wi, pi, loads

    for t in range(T_TOT):
        nodes = np.where(gtile == t)[0]
        V = C[nodes]
        order = np.argsort(-V.max(1), kind="stable")
        wi, pi, loads = pack_tile(V, order)
        tries = 0
        while (loads > 128).any() and tries < 60:
            noise = rng.random(len(V))
            order = np.argsort(-(V.max(1) + noise), kind="stable")
            wi, pi, loads = pack_tile(V, order)
            tries += 1
        if (loads > 128).any():
            return None, None
        win[nodes] = wi
        posn[nodes] = pi
    return win, posn


def _build_dir(key, gat, ew, gtile, win, gpart, H):
    """Bucket one direction's edges into per-(tile, half, window)
    128-slot blocks.

    Returns (gidx [2, T_TOT*H*128] int16, S [2, 128, T_TOT*128] f32)
    where for stream half h: block col = t*H + w, partition p = edge
    slot; S[h][slot, t*128 + gpart[key]] = e.
    """
    half = (gat >= HALF).astype(np.int64)
    t = gtile[key].astype(np.int64)
    w = win[key].astype(np.int64)
    cell = (half * T_TOT + t) * H + w
    order = np.argsort(cell, kind="stable")
    cell_s = cell[order]
    cnt = np.bincount(cell_s, minlength=2 * T_TOT * H)
    assert cnt.max() <= 128, (cnt.max(),)
    starts = np.zeros(2 * T_TOT * H, np.int64)
    starts[1:] = np.cumsum(cnt)[:-1]
    pos = np.arange(len(key)) - starts[cell_s]
    slot = cell_s * 128 + pos

    gidx = np.zeros(2 * T_TOT * H * 128, np.int16)
    gidx[slot] = (gat[order] - half[order] * HALF).astype(np.int16)
    gidx = gidx.reshape(2, T_TOT * H * 128)

    S = np.zeros((2, 128, T_TOT * 128), np.float32)
    ko = key[order]
    S[half[order], pos, t[order] * 128 + gpart[ko]] = ew[order]
    return gidx, S


def _wrap_idx(arr):
    """[L] int16 -> [128, L//16] in the dma_gather layout: idx i at
    [i % 16, i // 16], replicated across the 8 Q7 core stripes."""
    L = arr.shape[0]
    w = arr.reshape(L // 16, 16).T  # [16, L//16]
    return np.ascontiguousarray(np.tile(w, (8, 1)))


def _preprocess(x, e, edge_index):
    src = np.asarray(edge_index[0], np.int64)
    dst = np.asarray(edge_index[1], np.int64)
    ew = np.asarray(e, np.float32)
    xpad = np.zeros((NPAD, D), np.float32)
    xpad[:N] = np.asarray(x, np.float32)

    c1 = np.bincount(dst[src < HALF], minlength=NPAD)
    c2 = np.bincount(dst[src >= HALF], minlength=NPAD)
    c3 = np.bincount(src[dst < HALF], minlength=NPAD)
    c4 = np.bincount(src[dst >= HALF], minlength=NPAD)
    C = np.stack([c1, c2, c3, c4], 1)

    gtile = _snake_slots(C)
    H = 9
    win, posn = _pack_windows(C, gtile, H)
    while win is None:
        H += 1
        win, posn = _pack_windows(C, gtile, H)
    bounds, _ = _win_bounds(H)
    woff = np.array(bounds[:-1])
    gpart = (woff[win] + posn).astype(np.int32)

    gidx_mi, S_mi = _build_dir(dst, src, ew, gtile, win, gpart, H)
    gidx_mo, S_mo = _build_dir(src, dst, ew, gtile, win, gpart, H)

    # feature-major x in slot order for the MLP concat input
    perm_nodes = np.empty(NPAD, np.int64)
    gslot = gtile.astype(np.int64) * 128 + gpart
    perm_nodes[gslot] = np.arange(NPAD)
    xpermT = np.ascontiguousarray(xpad[perm_nodes].T)  # [128, NPAD]

    x_lo = xpad[:HALF].astype(bf16_np)
    x_hi = xpad[HALF:].astype(bf16_np)

    per_core = []
    for k in range(N_CORES):
        cs = slice(k * T_CORE * 128, (k + 1) * T_CORE * 128)
        bs = slice(k * T_CORE * H * 128, (k + 1) * T_CORE * H * 128)
        m = {
            "x_lo": x_lo,
            "x_hi": x_hi,
            "xT": np.ascontiguousarray(xpermT[:, cs]).astype(bf16_np),
        }
        for dname, gi, S in (("mi", gidx_mi, S_mi), ("mo", gidx_mo, S_mo)):
            for h in (0, 1):
                m[f"idx_{dname}{h}"] = _wrap_idx(gi[h, bs])
                m[f"S_{dname}{h}"] = np.ascontiguousarray(
                    S[h][:, cs]).astype(bf16_np)
        per_core.append(m)
    return per_core, gslot, H


_NC_CACHE = {}


def _build_nc(H):
    if H in _NC_CACHE:
        return _NC_CACHE[H]
    NBLK = T_CORE * H            # blocks per (dir, half) stream
    bounds, wsizes = _win_bounds(H)
    nc = bacc.Bacc("TRN2", target_bir_lowering=False, debug=False,
                   enable_asserts=False, num_devices=N_CORES,
                   num_swdge_queues=4)

    x_lo = nc.dram_tensor("x_lo", [HALF, D], bf16, kind="ExternalInput").ap()
    x_hi = nc.dram_tensor("x_hi", [HALF, D], bf16, kind="ExternalInput").ap()
    xT = nc.dram_tensor("xT", [128, T_CORE * 128], bf16,
                        kind="ExternalInput").ap()
    idx_d, S_d = {}, {}
    for dname in ("mi", "mo"):
        for h in (0, 1):
            idx_d[(dname, h)] = nc.dram_tensor(
                f"idx_{dname}{h}", [128, NBLK * 8], i16,
                kind="ExternalInput").ap()
            S_d[(dname, h)] = nc.dram_tensor(
                f"S_{dname}{h}", [128, T_CORE * 128], bf16,
                kind="ExternalInput").ap()
    w1 = nc.dram_tensor("W1", [3 * D, D], bf16, kind="ExternalInput").ap()
    wds = {2: nc.dram_tensor("W2", [D, D], bf16, kind="ExternalInput").ap(),
           3: nc.dram_tensor("W3", [D, D], bf16, kind="ExternalInput").ap(),
           4: nc.dram_tensor("W4", [D, D], bf16, kind="ExternalInput").ap()}
    bds = {i: nc.dram_tensor(f"b{i}", [D], f32, kind="ExternalInput").ap()
           for i in (1, 2, 3, 4)}
    out_t = nc.dram_tensor("out_t", [128, T_CORE * 128], bf16,
                           kind="ExternalOutput").ap()

    tanh = mybir.ActivationFunctionType.Tanh
    streams = [("mi", 0), ("mi", 1), ("mo", 0), ("mo", 1)]

    with tile.TileContext(nc) as tc:
        with (
            tc.tile_pool(name="const", bufs=1) as cpool,
            tc.tile_pool(name="gath", bufs=16) as gpool,
            tc.tile_pool(name="sload", bufs=2) as slpool,
            tc.tile_pool(name="hbuf", bufs=3) as hpool,
            tc.tile_pool(name="ps", bufs=4, space="PSUM") as pspool,
            tc.tile_pool(name="psm", bufs=2, space="PSUM") as mpool,
        ):
            # ---- bulk constant/metadata loads ----
            xt_all = cpool.tile([128, T_CORE * 128], bf16, tag="xt",
                                name="xt")
            nc.sync.dma_start(out=xt_all[:], in_=xT[:, :])
            obuf = cpool.tile([128, T_CORE * 128], bf16, tag="obuf",
                              name="obuf")
            idx_t = {}
            for s in streams:
                dname, h = s
                idx_t[s] = cpool.tile([128, NBLK * 8], i16,
                                      tag=f"idx{dname}{h}",
                                      name=f"idx{dname}{h}")
                nc.sync.dma_start(out=idx_t[s][:], in_=idx_d[s][:, :])
            wt = {}
            for j in range(3):
                wt[(1, j)] = cpool.tile([128, 128], bf16, tag=f"w1{j}",
                                        name=f"w1{j}")
                nc.sync.dma_start(out=wt[(1, j)][:],
                                  in_=w1[j * 128:(j + 1) * 128, :])
            for i in (2, 3, 4):
                wt[i] = cpool.tile([128, 128], bf16, tag=f"w{i}",
                                   name=f"w{i}")
                nc.sync.dma_start(out=wt[i][:], in_=wds[i][:, :])
            bt = {}
            for i in (1, 2, 3, 4):
                bt[i] = cpool.tile([128, 1], f32, tag=f"b{i}",
                                   name=f"b{i}")
                nc.sync.dma_start(out=bt[i][:], in_=bds[i][:, None])

            # ---- rolling S-group loads (SG tiles per group) ----
            n_sg = (T_CORE + SG - 1) // SG
            s_grp = {s: [None] * n_sg for s in streams}

            def load_sgroup(g, only_stream=None):
                if g >= n_sg:
                    return
                lo = g * SG * 128
                hi = min((g + 1) * SG, T_CORE) * 128
                for s in (streams if only_stream is None else [only_stream]):
                    dname, h = s
                    st = slpool.tile([128, SG * 128], bf16,
                                     tag=f"S{dname}{h}", name=f"S{dname}{h}")
                    nc.sync.dma_start(out=st[:, :hi - lo],
                                      in_=S_d[s][:, lo:hi])
                    s_grp[s][g] = st

            load_sgroup(0)

            # ---- rolling gather chunks (prepare/trigger pipelined) ----
            # num_idxs_reg via persistent registers: a fresh to_reg per
            # call emits a MOVE into one shared GPR, whose WAR hazard
            # against the previous gather serializes the whole gather
            # pipeline (measured 3-45us waits per MOVE). prepare_only
            # moves SWDGE desc-gen off the transfer chain: preps run
            # during earlier transfers; trigger_dma fires them with ~no
    
            # engine time, so transfers chain at the DMA-engine rate.
            nidx_regs = {}
            for nb in {CHUNK, NBLK - (NBLK - 1) // CHUNK * CHUNK}:
                r = nc.gpsimd.alloc_register(f"nidx{nb}")
                nc.gpsimd.reg_mov(r, nb * 128)
                nidx_regs[nb] = r
            # completion sems are locked to one SWDGE queue each
            qsem = []
            for q in range(4):
                qsem.append(nc.alloc_semaphore(f"gsem_q{q}"))
                nc.gpsimd.sem_clear(qsem[q])
            qcount = [0, 0, 0, 0]
            chunks = {s: [] for s in streams}   # chunk tiles per stream
            chunk_done = {s: [] for s in streams}  # (queue, 16*ordinal)
            next_chunk = {s: 0 for s in streams}
            waited_chunk = {s: -1 for s in streams}
            qrr = [0]

            def emit_chunks(upto_block):
                for s in streams:
                    dname, h = s
                    while (next_chunk[s] * CHUNK < upto_block
                           and next_chunk[s] * CHUNK < NBLK):
                        c = next_chunk[s]
                        nb = min(CHUNK, NBLK - c * CHUNK)
                        nidx = nb * 128
                        gb = gpool.tile([128, nb, 128], bf16,
                                        tag=f"g{dname}{h}",
                                        name=f"g{dname}{h}")
                        q = (qrr[0] + 1) % 4   # rotate 1,2,3,0,...
                        qrr[0] = q
                        nc.gpsimd.dma_gather(
                            out_ap=gb[:],
                            in_ap=(x_lo if h == 0 else x_hi)[:, :],
                            idxs_ap=idx_t[s][:, c * CHUNK * 8:
                                             (c * CHUNK + nb) * 8],
                            num_idxs=nidx,
                            num_idxs_reg=nidx_regs[nb],
                            elem_size=D,
                            single_packet=True,
                            queue_num=q,
                            prepare_only=True,
                            sem=qsem[q],
                        )
                        nc.gpsimd.trigger_dma(count=None, queue_num=q)
                        qcount[q] += 1
                        chunk_done[s].append((q, 16 * qcount[q]))
                        chunks[s].append(gb)
                        next_chunk[s] += 1

            accb = {
                dname: hpool.tile([128, GM * 128], bf16,
                                  tag=f"acc{dname}", name=f"acc{dname}")
                for dname in ("mi", "mo")}
            for t in range(T_CORE):
                emit_chunks(min((t + LA) * H, NBLK))
                if t % SG == 0:
                    load_sgroup(t // SG + 1)
                sgt = t // SG
                scol = (t % SG) * 128

                for dname in ("mi", "mo"):
                    ps = pspool.tile([128, 128], f32, tag="scat")
                    nc.vector.memset(ps[:], 0.0)
                    for h in (0, 1):
                        s = (dname, h)
                        for w in range(H):
                            blk = t * H + w
                            ch = blk // CHUNK
                            if ch > waited_chunk[s]:
                                cq, cnt = chunk_done[s][ch]
                                nc.tensor.wait_ge(qsem[cq], cnt)
                                waited_chunk[s] = ch
                            w0, w1c = bounds[w], bounds[w + 1]
                            y = chunks[s][ch][:, blk % CHUNK, :]
                            nc.tensor.matmul(
                                out=ps[:, w0:w1c], lhsT=y,
                                rhs=s_grp[s][sgt][:, scol + w0:scol + w1c],
                                start=False,
                                stop=(h == 1 and w == H - 1),
                                skip_group_check=True)
                    tg = t % GM
                    nc.scalar.copy(out=accb[dname][:, tg * 128:
                                                   (tg + 1) * 128],
                                   in_=ps[:])

                # batched MLP over GM tiles, feature-major bf16
                if t % GM == GM - 1 or t == T_CORE - 1:
                    W = (t % GM + 1) * 128
                    g0 = (t - t % GM) * 128
                    g1 = (t + 1) * 128
                    hp = mpool.tile([128, GM * 128], f32, tag="mlp")
                    nc.tensor.matmul(out=hp[:, :W], lhsT=wt[(1, 0)][:],
                                     rhs=accb["mi"][:, :W], start=True,
                                     stop=False)
                    nc.tensor.matmul(out=hp[:, :W], lhsT=wt[(1, 1)][:],
                                     rhs=accb["mo"][:, :W], start=False,
                                     stop=False)
                    nc.tensor.matmul(out=hp[:, :W], lhsT=wt[(1, 2)][:],
                                     rhs=xt_all[:, g0:g1],
                                     start=False, stop=True)
                    hprev = hpool.tile([128, GM * 128], bf16, tag="h")
                    nc.scalar.activation(hprev[:, :W], hp[:, :W], tanh,
                                         bias=bt[1][:, 0:1])
                    for i in (2, 3):
                        hp = mpool.tile([128, GM * 128], f32, tag="mlp")
                        nc.tensor.matmul(out=hp[:, :W], lhsT=wt[i][:],
                                         rhs=hprev[:, :W], start=True,
                                         stop=True)
                        hnext = hpool.tile([128, GM * 128], bf16, tag="h")
                        nc.scalar.activation(hnext[:, :W], hp[:, :W], tanh,
                                             bias=bt[i][:, 0:1])
                        hprev = hnext
                    hp = mpool.tile([128, GM * 128], f32, tag="mlp")
                    nc.tensor.matmul(out=hp[:, :W], lhsT=wt[4][:],
                                     rhs=hprev[:, :W], start=True,
                                     stop=True)
                    nc.scalar.activation(obuf[:, g0:g1],
                                         hp[:, :W], tanh, bias=bt[4][:, 0:1])
                    nc.sync.dma_start(out=out_t[:, g0:g1],
                                      in_=obuf[:, g0:g1])
                    accb = {
                        dname: hpool.tile([128, GM * 128], bf16,
                                          tag=f"acc{dname}",
                                          name=f"acc{dname}")
                        for dname in ("mi", "mo")}

    nc.compile()
    _NC_CACHE[H] = nc
    return nc


def kernel(**inputs):
    global LAST_RESULTS
    _register_ntff_hook()
    x = np.asarray(inputs["x"], np.float32)
    e = np.asarray(inputs["e"], np.float32)
    edge_index = np.asarray(inputs["edge_index"])

    per_core, gslot, H = _preprocess(x, e, edge_index)
    nc = _build_nc(H)

    shared = {}
    for i in (1, 2, 3, 4):
        shared[f"W{i}"] = np.asarray(inputs[f"W{i}"],
                                     np.float32).astype(bf16_np)
        shared[f"b{i}"] = np.asarray(inputs[f"b{i}"], np.float32)

    in_maps = []
    for k in range(N_CORES):
        m = dict(per_core[k])
        m.update(shared)
        in_maps.append(m)

    res = bass_utils.run_bass_kernel_spmd(nc, in_maps,
                                          core_ids=list(range(N_CORES)))
    LAST_RESULTS = res
    big = np.concatenate([np.asarray(res.results[k]["out_t"])
                          .astype(np.float32)
                          for k in range(N_CORES)],
                         axis=1)  # [128, NPAD] feature-major, slot order
    out = big.T[gslot[:N]]
    return np.ascontiguousarray(out.astype(np.float32))


# revision 28
# speedup vs baseline: 4.5614x; 4.5614x over previous
"""GNN message-passing kernel for 8 Trainium2 NeuronCores.

Strategy (node-sharded, zero collectives), v5 (bf16, host-built
selection matrices):
  - Pad nodes to 50176 = 8 cores x 49 tiles x 128 slots. A host-side
    "snake deal" assigns nodes to tiles balancing per-tile edge counts;
    within each tile, nodes are packed into H fixed-size windows (~13
    nodes each) such that every window's edge count is <= 128 in all 4
    streams (mi/mo x lo/hi half). The node's partition within the tile
    is its window offset + position (the MLP is pointwise, so any node
    permutation is legal; it is undone on the host at the end).
  - Edges are duplicated per direction: mi keyed by dst (gather x[src]),
    mo keyed by src (gather x[dst]); per (tile, half) they are grouped
    by the key node's window into H blocks of <= 128 edges (one padded
    128-slot block per window).
  - On-core: rolling dma_gather chunks bring bf16 x rows into SBUF.
    Aggregation per block is a single PE matmul psum[:, w0:w1] +=
    Y^T @ S where S [128 slots, ~13 window nodes] is a host-built bf16
    selection matrix carrying the edge weights - no on-chip one-hot
    construction at all (v2-v3 burned 300-400ns of DVE per block there).
    Window columns are disjoint per stream, so lo-half blocks write
    with start=True and hi-half blocks accumulate.
  - All metadata (gather indices, xT) is bulk-loaded up front; S is
    streamed in 8-tile double-buffered groups; output is buffered in
    SBUF (bf16) and stored in 4-tile groups.
  - The 4-layer MLP runs feature-major in bf16, batched over 4 tiles
    per matmul group (one psum bank wide).
"""

import os
import sys

sys.path.insert(0, "/opt/trn_rl_repo")

import numpy as np
import ml_dtypes

from concourse import bass, bacc, mybir, tile
from concourse import bass_utils

N = 50000
E = 800000
D = 128
N_CORES = 8
T_CORE = 49                      # tiles per core
T_TOT = N_CORES * T_CORE         # 392 tiles
NPAD = T_TOT * 128               # 50176
HALF = NPAD // 2                 # 25088 (int16 index limit is 32767)
CHUNK = 8                        # gather blocks per dma_gather call
LA = 8                           # gather lookahead in tiles
GM = 4                           # tiles per batched-MLP group (psum bank cap)
SG = 4                           # tiles per rolling S-load group

bf16_np = ml_dtypes.bfloat16

f32 = mybir.dt.float32
bf16 = mybir.dt.bfloat16
i16 = mybir.dt.int16

LAST_RESULTS = None              # BassKernelResults of the last run


def _register_ntff_hook():
    """Make trace=True work under axon by registering the NTFF profile
    hook that the agent image's antenv package lacks."""
    import types, ctypes, contextlib

    if "antenv.axon_hooks" in sys.modules:
        return
    so_path = "/opt/axon/libaxon_pjrt.so"
    if not os.path.exists(so_path):
        return
    try:
        lib = ctypes.CDLL(so_path)
        if not hasattr(lib, "axon_start_nrt_profile"):
            return
        lib.axon_start_nrt_profile.argtypes = [
            ctypes.POINTER(ctypes.c_int64), ctypes.c_size_t]
        lib.axon_start_nrt_profile.restype = ctypes.c_int64
        lib.axon_stop_nrt_profile.argtypes = [ctypes.c_char_p]
        lib.axon_stop_nrt_profile.restype = ctypes.c_int64

        @contextlib.contextmanager
        def _hook(output_dir, device_ids):
            import jax
            jax.devices()
            if device_ids:
                ids = (ctypes.c_int64 * len(device_ids))(*device_ids)
                rc = lib.axon_start_nrt_profile(ids, len(device_ids))
            else:
                rc = lib.axon_start_nrt_profile(None, 0)
            if rc != 0:
                raise RuntimeError(f"axon_start_nrt_profile rc={rc}")
            try:
                yield
            finally:
                n = lib.axon_stop_nrt_profile(str(output_dir).encode())
                print(f"profile: {n} file(s) -> {output_dir}", file=sys.stderr)

        mod = types.ModuleType("antenv.axon_hooks")
        mod.get_axon_ntff_profile_hook = lambda: _hook
        sys.modules["antenv.axon_hooks"] = mod
    except OSError:
        pass


def _win_bounds(H):
    b = [round(128 * i / H) for i in range(H + 1)]
    return b, [b[i + 1] - b[i] for i in range(H)]


def _snake_slots(C):
    """Assign each padded node to a global tile, balancing total edge
    counts across tiles (snake deal on the 4-cell sum)."""
    tot = C.sum(1)
    rank = np.argsort(-tot, kind="stable")
    seq = np.arange(NPAD)
    rounds = seq // T_TOT
    k = seq % T_TOT
    tile_seq = np.where(rounds % 2 == 0, k, T_TOT - 1 - k).astype(np.int32)
    gtile = np.empty(NPAD, np.int32)
    gtile[rank] = tile_seq
    return gtile


def _pack_windows(C, gtile, H):
    """Within each tile, pack its 128 nodes into H fixed-size windows
    so that every window's edge count is <= 128 in each of the 4
    streams (greedy with randomized restarts). Returns (win, posn) per
    node, or (None, None) if packing fails for any tile."""
    bounds, sizes = _win_bounds(H)
    sizes = np.array(sizes)
    win = np.empty(NPAD, np.int32)
    posn = np.empty(NPAD, np.int32)
    rng = np.random.default_rng(7)

    def pack_tile(V, order):
        loads = np.zeros((H, 4), np.int64)
        cnt = np.zeros(H, np.int64)
        wi = np.empty(len(V), np.int32)
        pi = np.empty(len(V), np.int32)
        for i in order:
            v = V[i]
            sc = (loads + v).max(1) + ((loads + v) > 128).any(1) * 100000
            sc = np.where(cnt >= sizes, 10 ** 9, sc)
            b = int(np.argmin(sc))
            wi[i] = b
            pi[i] = cnt[b]
            loads[b] += v
            cnt[b] += 1
        return wi, pi, loads

    for t in range(T_TOT):
        nodes = np.where(gtile == t)[0]
        V = C[nodes]
        order = np.argsort(-V.max(1), kind="stable")
        wi, pi, loads = pack_tile(V, order)
        tries = 0
        while (loads > 128).any() and tries < 60:
            noise = rng.random(len(V))
            order = np.argsort(-(V.max(1) + noise), kind="stable")
            wi, pi, loads = pack_tile(V, order)
            tries += 1
        if (loads > 128).any():
            return None, None
        win[nodes] = wi
        posn[nodes] = pi
    return win, posn


def _build_dir(key, gat, ew, gtile, win, gpart, H):
    """Bucket one direction's edges into per-(tile, half, window)
    128-slot blocks.

    Returns (gidx [2, T_TOT*H*128] int16, S [2, 128, T_TOT*128] f32)
    where for stream half h: block col = t*H + w, partition p = edge
    slot; S[h][slot, t*128 + gpart[key]] = e.
    """
    half = (gat >= HALF).astype(np.int64)
    t = gtile[key].astype(np.int64)
    w = win[key].astype(np.int64)
    cell = (half * T_TOT + t) * H + w
    order = np.argsort(cell, kind="stable")
    cell_s = cell[order]
    cnt = np.bincount(cell_s, minlength=2 * T_TOT * H)
    assert cnt.max() <= 128, (cnt.max(),)
    starts = np.zeros(2 * T_TOT * H, np.int64)
    starts[1:] = np.cumsum(cnt)[:-1]
    pos = np.arange(len(key)) - starts[cell_s]
    slot = cell_s * 128 + pos

    gidx = np.zeros(2 * T_TOT * H * 128, np.int16)
    gidx[slot] = (gat[order] - half[order] * HALF).astype(np.int16)
    gidx = gidx.reshape(2, T_TOT * H * 128)

    S = np.zeros((2, 128, T_TOT * 128), np.float32)
    ko = key[order]
    S[half[order], pos, t[order] * 128 + gpart[ko]] = ew[order]
    return gidx, S


def _wrap_idx(arr):
    """[L] int16 -> [128, L//16] in the dma_gather layout: idx i at
    [i % 16, i // 16], replicated across the 8 Q7 core stripes."""
    L = arr.shape[0]
    w = arr.reshape(L // 16, 16).T  # [16, L//16]
    return np.ascontiguousarray(np.tile(w, (8, 1)))


def _preprocess(x, e, edge_index):
    src = np.asarray(edge_index[0], np.int64)
    dst = np.asarray(edge_index[1], np.int64)
    ew = np.asarray(e, np.float32)
    xpad = np.zeros((NPAD, D), np.float32)
    xpad[:N] = np.asarray(x, np.float32)

    c1 = np.bincount(dst[src < HALF], minlength=NPAD)
    c2 = np.bincount(dst[src >= HALF], minlength=NPAD)
    c3 = np.bincount(src[dst < HALF], minlength=NPAD)
    c4 = np.bincount(src[dst >= HALF], minlength=NPAD)
    C = np.stack([c1, c2, c3, c4], 1)

    gtile = _snake_slots(C)
    H = 9
    win, posn = _pack_windows(C, gtile, H)
    while win is None:
        H += 1
        win, posn = _pack_windows(C, gtile, H)
    bounds, _ = _win_bounds(H)
    woff = np.array(bounds[:-1])
    gpart = (woff[win] + posn).astype(np.int32)

    gidx_mi, S_mi = _build_dir(dst, src, ew, gtile, win, gpart, H)
    gidx_mo, S_mo = _build_dir(src, dst, ew, gtile, win, gpart, H)

    # feature-major x in slot order for the MLP concat input
    perm_nodes = np.empty(NPAD, np.int64)
    gslot = gtile.astype(np.int64) * 128 + gpart
    perm_nodes[gslot] = np.arange(NPAD)
    xpermT = np.ascontiguousarray(xpad[perm_nodes].T)  # [128, NPAD]

    x_lo = xpad[:HALF].astype(bf16_np)
    x_hi = xpad[HALF:].astype(bf16_np)

    per_core = []
    for k in range(N_CORES):
        cs = slice(k * T_CORE * 128, (k + 1) * T_CORE * 128)
        bs = slice(k * T_CORE * H * 128, (k + 1) * T_CORE * H * 128)
        m = {
            "x_lo": x_lo,
            "x_hi": x_hi,
            "xT": np.ascontiguousarray(xpermT[:, cs]).astype(bf16_np),
        }
        for dname, gi, S in (("mi", gidx_mi, S_mi), ("mo", gidx_mo, S_mo)):
            for h in (0, 1):
                m[f"idx_{dname}{h}"] = _wrap_idx(gi[h, bs])
                m[f"S_{dname}{h}"] = np.ascontiguousarray(
                    S[h][:, cs]).astype(bf16_np)
        per_core.append(m)
    return per_core, gslot, H


_NC_CACHE = {}


def _build_nc(H):
    if H in _NC_CACHE:
        return _NC_CACHE[H]
    NBLK = T_CORE * H            # blocks per (dir, half) stream
    bounds, wsizes = _win_bounds(H)
    nc = bacc.Bacc("TRN2", target_bir_lowering=False, debug=False,
                   enable_asserts=False, num_devices=N_CORES,
                   num_swdge_queues=4)

    x_lo = nc.dram_tensor("x_lo", [HALF, D], bf16, kind="ExternalInput").ap()
    x_hi = nc.dram_tensor("x_hi", [HALF, D], bf16, kind="ExternalInput").ap()
    xT = nc.dram_tensor("xT", [128, T_CORE * 128], bf16,
                        kind="ExternalInput").ap()
    idx_d, S_d = {}, {}
    for dname in ("mi", "mo"):
        for h in (0, 1):
            idx_d[(dname, h)] = nc.dram_tensor(
                f"idx_{dname}{h}", [128, NBLK * 8], i16,
                kind="ExternalInput").ap()
            S_d[(dname, h)] = nc.dram_tensor(
                f"S_{dname}{h}", [128, T_CORE * 128], bf16,
                kind="ExternalInput").ap()
    w1 = nc.dram_tensor("W1", [3 * D, D], bf16, kind="ExternalInput").ap()
    wds = {2: nc.dram_tensor("W2", [D, D], bf16, kind="ExternalInput").ap(),
           3: nc.dram_tensor("W3", [D, D], bf16, kind="ExternalInput").ap(),
           4: nc.dram_tensor("W4", [D, D], bf16, kind="ExternalInput").ap()}
    bds = {i: nc.dram_tensor(f"b{i}", [D], f32, kind="ExternalInput").ap()
           for i in (1, 2, 3, 4)}
    out_t = nc.dram_tensor("out_t", [128, T_CORE * 128], bf16,
                           kind="ExternalOutput").ap()

    tanh = mybir.ActivationFunctionType.Tanh
    streams = [("mi", 0), ("mi", 1), ("mo", 0), ("mo", 1)]

    with tile.TileContext(nc) as tc:
        with (
            tc.tile_pool(name="const", bufs=1) as cpool,
            tc.tile_pool(name="gath", bufs=16) as gpool,
            tc.tile_pool(name="sload", bufs=2) as slpool,
            tc.tile_pool(name="hbuf", bufs=4) as hpool,
            tc.tile_pool(name="ps", bufs=6, space="PSUM") as pspool,
            tc.tile_pool(name="psm", bufs=2, space="PSUM") as mpool,
        ):
            # ---- bulk constant/metadata loads ----
            xt_all = cpool.tile([128, T_CORE * 128], bf16, tag="xt",
                                name="xt")
            nc.sync.dma_start(out=xt_all[:], in_=xT[:, :])
            obuf = cpool.tile([128, T_CORE * 128], bf16, tag="obuf",
                              name="obuf")
            idx_t = {}
            for s in streams:
                dname, h = s
                idx_t[s] = cpool.tile([128, NBLK * 8], i16,
                                      tag=f"idx{dname}{h}",
                                      name=f"idx{dname}{h}")
                nc.sync.dma_start(out=idx_t[s][:], in_=idx_d[s][:, :])
            wt = {}
            for j in range(3):
                wt[(1, j)] = cpool.tile([128, 128], bf16, tag=f"w1{j}",
                                        name=f"w1{j}")
                nc.sync.dma_start(out=wt[(1, j)][:],
                                  in_=w1[j * 128:(j + 1) * 128, :])
            for i in (2, 3, 4):
                wt[i] = cpool.tile([128, 128], bf16, tag=f"w{i}",
                                   name=f"w{i}")
                nc.sync.dma_start(out=wt[i][:], in_=wds[i][:, :])
            bt = {}
            for i in (1, 2, 3, 4):
                bt[i] = cpool.tile([128, 1], f32, tag=f"b{i}",
                                   name=f"b{i}")
                nc.sync.dma_start(out=bt[i][:], in_=bds[i][:, None])

            # ---- rolling S-group loads (SG tiles per group) ----
            n_sg = (T_CORE + SG - 1) // SG
            s_grp = {s: [None] * n_sg for s in streams}

            def load_sgroup(g, only_stream=None):
                if g >= n_sg:
                    return
                lo = g * SG * 128
                hi = min((g + 1) * SG, T_CORE) * 128
                for s in (streams if only_stream is None else [only_stream]):
                    dname, h = s
                    st = slpool.tile([128, SG * 128], bf16,
                                     tag=f"S{dname}{h}", name=f"S{dname}{h}")
                    nc.sync.dma_start(out=st[:, :hi - lo],
                                      in_=S_d[s][:, lo:hi])
                    s_grp[s][g] = st

            load_sgroup(0)

            # ---- rolling gather chunks ----
            # num_idxs_reg via persistent registers: a fresh to_reg per
            # call emits a MOVE into one shared GPR, whose WAR hazard
            # against the previous gather serializes the whole gather
            # pipeline (measured 3-45us waits per MOVE).
            nidx_regs = {}
            for nb in {CHUNK, NBLK - (NBLK - 1) // CHUNK * CHUNK}:
                r = nc.gpsimd.alloc_register(f"nidx{nb}")
                nc.gpsimd.reg_mov(r, nb * 128)
                nidx_regs[nb] = r
            chunks = {s: [] for s in streams}   # chunk tiles per stream
            next_chunk = {s: 0 for s in streams}
            qrr = [0]

            def emit_chunks(upto_block):
                for s in streams:
                    dname, h = s
                    while (next_chunk[s] * CHUNK < upto_block
                           and next_chunk[s] * CHUNK < NBLK):
                        c = next_chunk[s]
                        nb = min(CHUNK, NBLK - c * CHUNK)
                        nidx = nb * 128
                        gb = gpool.tile([128, nb, 128], bf16,
                                        tag=f"g{dname}{h}",
                                        name=f"g{dname}{h}")
                        q = (qrr[0] + 1) % 4   # rotate 1,2,3,0,...
                        qrr[0] = q
                        nc.gpsimd.dma_gather(
                            out_ap=gb[:],
                            in_ap=(x_lo if h == 0 else x_hi)[:, :],
                            idxs_ap=idx_t[s][:, c * CHUNK * 8:
                                             (c * CHUNK + nb) * 8],
                            num_idxs=nidx,
                            num_idxs_reg=nidx_regs[nb],
                            elem_size=D,
                            single_packet=True,
                            queue_num=q,
                        )
                        chunks[s].append(gb)
                        next_chunk[s] += 1

            accb = {
                dname: hpool.tile([128, GM * 128], bf16,
                                  tag=f"acc{dname}", name=f"acc{dname}")
                for dname in ("mi", "mo")}
            for t in range(T_CORE):
                emit_chunks(min((t + LA) * H, NBLK))
                if t % SG == 0:
                    load_sgroup(t // SG + 1)
                sgt = t // SG
                scol = (t % SG) * 128

                for dname in ("mi", "mo"):
                    ps = pspool.tile([128, 128], f32, tag="scat")
                    nc.vector.memset(ps[:], 0.0)
                    for h in (0, 1):
                        s = (dname, h)
                        for w in range(H):
                            blk = t * H + w
                            w0, w1c = bounds[w], bounds[w + 1]
                            y = chunks[s][blk // CHUNK][:, blk % CHUNK, :]
                            nc.tensor.matmul(
                                out=ps[:, w0:w1c], lhsT=y,
                                rhs=s_grp[s][sgt][:, scol + w0:scol + w1c],
                                start=False,
                                stop=(h == 1 and w == H - 1),
                                skip_group_check=True)
                    tg = t % GM
                    nc.scalar.copy(out=accb[dname][:, tg * 128:
                                                   (tg + 1) * 128],
                                   in_=ps[:])

                # batched MLP over GM tiles, feature-major bf16
                if t % GM == GM - 1 or t == T_CORE - 1:
                    W = (t % GM + 1) * 128
                    g0 = (t - t % GM) * 128
                    g1 = (t + 1) * 128
                    hp = mpool.tile([128, GM * 128], f32, tag="mlp")
                    nc.tensor.matmul(out=hp[:, :W], lhsT=wt[(1, 0)][:],
                                     rhs=accb["mi"][:, :W], start=True,
                                     stop=False)
                    nc.tensor.matmul(out=hp[:, :W], lhsT=wt[(1, 1)][:],
                                     rhs=accb["mo"][:, :W], start=False,
                                     stop=False)
                    nc.tensor.matmul(out=hp[:, :W], lhsT=wt[(1, 2)][:],
                                     rhs=xt_all[:, g0:g1],
                                     start=False, stop=True)
                    hprev = hpool.tile([128, GM * 128], bf16, tag="h")
                    nc.scalar.activation(hprev[:, :W], hp[:, :W], tanh,
                                         bias=bt[1][:, 0:1])
                    for i in (2, 3):
                        hp = mpool.tile([128, GM * 128], f32, tag="mlp")
                        nc.tensor.matmul(out=hp[:, :W], lhsT=wt[i][:],
                                         rhs=hprev[:, :W], start=True,
                                         stop=True)
                        hnext = hpool.tile([128, GM * 128], bf16, tag="h")
                        nc.scalar.activation(hnext[:, :W], hp[:, :W], tanh,
                                             bias=bt[i][:, 0:1])
                        hprev = hnext
                    hp = mpool.tile([128, GM * 128], f32, tag="mlp")
                    nc.tensor.matmul(out=hp[:, :W], lhsT=wt[4][:],
                                     rhs=hprev[:, :W], start=True,
                                     stop=True)
                    nc.scalar.activation(obuf[:, g0:g1],
                                         hp[:, :W], tanh, bias=bt[4][:, 0:1])
                    nc.sync.dma_start(out=out_t[:, g0:g1],
                                      in_=obuf[:, g0:g1])
                    accb = {
                        dname: hpool.tile([128, GM * 128], bf16,
                                          tag=f"acc{dname}",
                                          name=f"acc{dname}")
                        for dname in ("mi", "mo")}

    nc.compile()
    _NC_CACHE[H] = nc
    return nc


def kernel(**inputs):
    global LAST_RESULTS
    _register_ntff_hook()
    x = np.asarray(inputs["x"], np.float32)
    e = np.asarray(inputs["e"], np.float32)
    edge_index = np.asarray(inputs["edge_index"])

    per_core, gslot, H = _preprocess(x, e, edge_index)
    nc = _build_nc(H)

    shared = {}
    for i in (1, 2, 3, 4):
        shared[f"W{i}"] = np.asarray(inputs[f"W{i}"],
                                     np.float32).astype(bf16_np)
        shared[f"b{i}"] = np.asarray(inputs[f"b{i}"], np.float32)

    in_maps = []
    for k in range(N_CORES):
        m = dict(per_core[k])
        m.update(shared)
        in_maps.append(m)

    res = bass_utils.run_bass_kernel_spmd(nc, in_maps,
                                          core_ids=list(range(N_CORES)))
    LAST_RESULTS = res
    big = np.concatenate([np.asarray(res.results[k]["out_t"])
                          .astype(np.float32)
                          for k in range(N_CORES)],
                         axis=1)  # [128, NPAD] feature-major, slot order
    out = big.T[gslot[:N]]
    return np.ascontiguousarray(out.astype(np.float32))
